# revision 5
# baseline (speedup 1.0000x reference)
"""MultiHeadAttention Trainium2 kernel (8-core SPMD, full I/O).

Sharding: core c handles batch b = c // 4 and heads (2*(c%4), 2*(c%4)+1).
Each core computes:
  - attn[b, h0:h0+2]  (fp32, written directly)
  - y = partial output projection (summed on host across the 4 cores of b)

Device-side design per core (Bass/Tile):
  phase0: project qhT/khT [128, S] (head-dim on partitions, both heads) and
          vh [S, 128] from transposed inputs streamed from DRAM.
  A-pass: scores[s_q, s_k] = qhT.T @ khT (fp32r matmuls) + additive fp8 mask
          via identity-matmul into the same PSUM accumulation group;
          exp on ACT with accum_out giving row sums for free; normalize with
          per-partition reciprocal on DVE; DMA attn out.
  T-pass: transposed scores[s_k, s_q] the same way (masked with transposed
          fp8 mask); exp; ctxE^T[64, s_q] accumulated in PSUM over k-chunks.
  y-pass: y_h = (ctxE_h^T).T @ Wo_h per head, scaled by the per-row reciprocal
          (per-partition scale on ACT) and summed across the 2 heads.
Softmax skips the row-max subtraction: scores are O(1) (no overflow) and
masked entries are -10240 (exp underflows to exactly 0.0), so the result is
mathematically identical to the reference.
"""

import sys

for _p in ("/opt/trn_rl_repo", "/root/.axon_site/_ro/trn_rl_repo"):
    if _p not in sys.path:
        sys.path.append(_p)

import numpy as np
import ml_dtypes

import concourse.bass as bass
import concourse.tile as tile
from concourse import bacc, mybir
from concourse import bass_utils

F32 = mybir.dt.float32
F32R = mybir.dt.float32r
F8 = mybir.dt.float8e5
AF = mybir.ActivationFunctionType

S = 2048
D = 512
NC_CHUNKS = 16  # S / 128
KD = 4  # D / 128
MASK_NEG = -10240.0  # exactly representable in fp8e5m2; exp() underflows to 0


def _build_module():
    nc = bacc.Bacc("TRN2", target_bir_lowering=False, debug=False, num_devices=8)

    qT = nc.dram_tensor("qT", [D, S], F32R, kind="ExternalInput").ap()
    kT = nc.dram_tensor("kT", [D, S], F32R, kind="ExternalInput").ap()
    vT = nc.dram_tensor("vT", [D, S], F32R, kind="ExternalInput").ap()
    maskA = nc.dram_tensor("maskA", [S, S], F8, kind="ExternalInput").ap()
    maskT = nc.dram_tensor("maskT", [S, S], F8, kind="ExternalInput").ap()
    wq = nc.dram_tensor("wq", [D, 128], F32R, kind="ExternalInput").ap()
    wk = nc.dram_tensor("wk", [D, 128], F32R, kind="ExternalInput").ap()
    wv = nc.dram_tensor("wv", [D, 128], F32R, kind="ExternalInput").ap()
    wo = nc.dram_tensor("wo", [128, D], F32R, kind="ExternalInput").ap()
    bq = nc.dram_tensor("bq", [128, 1], F32, kind="ExternalInput").ap()
    bk = nc.dram_tensor("bk", [128, 1], F32, kind="ExternalInput").ap()
    ident = nc.dram_tensor("ident", [128, 128], F8, kind="ExternalInput").ap()

    attn2 = nc.dram_tensor("attn2", [2, S, S], F32, kind="ExternalOutput").ap()
    y = nc.dram_tensor("y", [S, D], F32, kind="ExternalOutput").ap()

    with tile.TileContext(nc) as tc:
        _emit(nc, tc, qT, kT, vT, maskA, maskT, wq, wk, wv, wo, bq, bk, ident,
              attn2, y)

    nc.compile()
    return nc


def _emit(nc, tc, qT, kT, vT, maskA, maskT, wq, wk, wv, wo, bq, bk, ident,
          attn2, y):
    with tc.tile_pool(name="persist", bufs=1) as persist:
        # ---- constants ----
        wq_sb = persist.tile([128, KD, 128], F32R)
        wk_sb = persist.tile([128, KD, 128], F32R)
        wv_sb = persist.tile([128, KD, 128], F32R)
        for w_sb, w in ((wq_sb, wq), (wk_sb, wk), (wv_sb, wv)):
            nc.sync.dma_start(out=w_sb, in_=w.rearrange("(k p) m -> p k m", p=128))
        wo_sb = persist.tile([128, D], F32R)
        nc.sync.dma_start(out=wo_sb, in_=wo)
        bq_sb = persist.tile([128, 1], F32)
        nc.sync.dma_start(out=bq_sb, in_=bq)
        bk_sb = persist.tile([128, 1], F32)
        nc.sync.dma_start(out=bk_sb, in_=bk)
        ident_sb = persist.tile([128, 128], F8)
        nc.sync.dma_start(out=ident_sb, in_=ident)

        # persistent activations
        qhT = persist.tile([128, S], F32R)  # both heads, head-dim on partitions
        khT = persist.tile([128, S], F32R)
        vh = persist.tile([128, NC_CHUNKS, 128], F32R)  # [s in chunk, chunk, dim]
        ctx_sb = persist.tile([128, S], F32R)  # ctxE^T both heads
        r_all = persist.tile([128, 32], F32)  # 1/rowsum per (head, q-chunk)

        # ---- phase 0: projections ----
        with tc.tile_pool(name="p0in", bufs=2) as p0in, \
             tc.tile_pool(name="p0ps", bufs=1, space="PSUM") as p0ps:
            pq = p0ps.tile([128, S], F32)  # 4 banks
            pk = p0ps.tile([128, S], F32)  # 4 banks
            for kc in range(KD):
                qt_k = p0in.tile([128, S], F32R, tag="qt")
                kt_k = p0in.tile([128, S], F32R, tag="kt")
                nc.sync.dma_start(out=qt_k, in_=qT[kc * 128:(kc + 1) * 128, :])
                nc.sync.dma_start(out=kt_k, in_=kT[kc * 128:(kc + 1) * 128, :])
                for n in range(4):
                    ns = slice(n * 512, (n + 1) * 512)
                    nc.tensor.matmul(pq[:, ns], wq_sb[:, kc, :],
                                     qt_k[:, ns],
                                     start=(kc == 0), stop=(kc == KD - 1))
                    nc.tensor.matmul(pk[:, ns], wk_sb[:, kc, :],
                                     kt_k[:, ns],
                                     start=(kc == 0), stop=(kc == KD - 1))
            for n in range(4):
                ns = slice(n * 512, (n + 1) * 512)
                # qh is pre-scaled by 1/sqrt(d_k); bq arrives pre-scaled too
                nc.scalar.activation(qhT[:, ns], pq[:, ns], AF.Identity,
                                     bias=bq_sb, scale=0.125)
                nc.scalar.activation(khT[:, ns], pk[:, ns], AF.Identity,
                                     bias=bk_sb, scale=1.0)

        with tc.tile_pool(name="p0v", bufs=3) as p0v, \
             tc.tile_pool(name="p0vps", bufs=2, space="PSUM") as p0vps:
            vT_r = vT.rearrange("(k p) (c m) -> c p k m", p=128, m=128)
            for sc in range(NC_CHUNKS):
                vt_sc = p0v.tile([128, KD, 128], F32R, tag="vt")
                nc.sync.dma_start(out=vt_sc, in_=vT_r[sc])
                pv = p0vps.tile([128, 128], F32, tag="pv")
                for kc in range(KD):
                    nc.tensor.matmul(pv, vt_sc[:, kc, :], wv_sb[:, kc, :],
                                     start=(kc == 0), stop=(kc == KD - 1))
                nc.scalar.activation(vh[:, sc, :], pv, AF.Copy)

        # ---- A pass: attn output ----
        with tc.tile_pool(name="amask", bufs=3) as amask, \
             tc.tile_pool(name="ea", bufs=2) as eap, \
             tc.tile_pool(name="attnst", bufs=2) as attnst, \
             tc.tile_pool(name="accp", bufs=2) as accp, \
             tc.tile_pool(name="aps", bufs=2, space="PSUM") as aps:
            for qc in range(NC_CHUNKS):
                qs128 = slice(qc * 128, (qc + 1) * 128)
                ma = amask.tile([128, S], F8, tag="ma")
                nc.sync.dma_start(out=ma, in_=maskA[qs128, :])
                for h in range(2):
                    hh = h * 64
                    hs = slice(hh, hh + 64)
                    pa = aps.tile([128, S], F32, tag="pa")  # 4 banks x2 bufs
                    for j in range(4):
                        ks = slice(j * 512, (j + 1) * 512)
                        nc.tensor.matmul(pa[:, ks], qhT[hs, qs128],
                                         khT[hs, ks], start=True, stop=False)
                        nc.tensor.matmul(pa[:, ks], ident_sb, ma[:, ks],
                                         start=False, stop=True,
                                         skip_group_check=True)
                    ea = eap.tile([128, S], F32, tag=f"ea{h}")
                    acc = accp.tile([128, 1], F32, tag=f"acc{h}")
                    nc.scalar.activation(ea, pa, AF.Exp, accum_out=acc)
                    idx = h * NC_CHUNKS + qc
                    nc.vector.reciprocal(r_all[:, idx:idx + 1], acc)
                    attn_sb = attnst.tile([128, S], F32, tag=f"attn{h}")
                    nc.vector.tensor_scalar_mul(attn_sb, ea, r_all[:, idx:idx + 1])
                    nc.sync.dma_start(out=attn2[h, qs128, :], in_=attn_sb)

        # ---- T pass: ctxE^T accumulation ----
        # Two s_q-half sweeps so everything fits in 8 PSUM banks. The ctx
        # matmul uses the full [128,128] vh as stationary operand: for head h
        # only partitions [64h, 64h+64) of the product are meaningful (the
        # other 64 are ignored), which keeps the PSUM dst partition at 0.
        with tc.tile_pool(name="tmask", bufs=3) as tmask, \
             tc.tile_pool(name="et", bufs=2) as etp, \
             tc.tile_pool(name="tps", bufs=2, space="PSUM") as tps, \
             tc.tile_pool(name="ctxps", bufs=1, space="PSUM") as ctxps:
            for half in range(2):
                hsl = slice(half * 1024, (half + 1) * 1024)
                pctx = [ctxps.tile([128, 1024], F32, tag="pctx0", name="pctx0"),
                        ctxps.tile([128, 1024], F32, tag="pctx1", name="pctx1")]
                for kc in range(NC_CHUNKS):
                    ks128 = slice(kc * 128, (kc + 1) * 128)
                    mt = tmask.tile([128, 1024], F8, tag="mt")
                    nc.sync.dma_start(out=mt, in_=maskT[ks128, hsl])
                    for h in range(2):
                        hh = h * 64
                        hs = slice(hh, hh + 64)
                        et = etp.tile([128, 1024], F32R, tag=f"et{h}")
                        pt = tps.tile([128, 1024], F32, tag="pt")  # 2 banks x2
                        for j in range(2):
                            q0 = half * 1024 + j * 512
                            qs = slice(q0, q0 + 512)
                            js = slice(j * 512, (j + 1) * 512)
                            nc.tensor.matmul(pt[:, js], khT[hs, ks128],
                                             qhT[hs, qs],
                                             start=True, stop=False)
                            nc.tensor.matmul(pt[:, js], ident_sb, mt[:, js],
                                             start=False, stop=True,
                                             skip_group_check=True)
                        nc.scalar.activation(et, pt, AF.Exp)
                        for j in range(2):
                            js = slice(j * 512, (j + 1) * 512)
                            nc.tensor.matmul(pctx[h][:, js], vh[:, kc, :],
                                             et[:, js],
                                             start=(kc == 0),
                                             stop=(kc == NC_CHUNKS - 1),
                                             skip_group_check=True)
                nc.scalar.activation(ctx_sb[0:64, hsl], pctx[0][0:64, :], AF.Copy)
                nc.scalar.activation(ctx_sb[64:128, hsl], pctx[1][64:128, :],
                                     AF.Copy)

        # ---- y pass: output projection with deferred softmax normalization ----
        with tc.tile_pool(name="yst", bufs=3) as yst, \
             tc.tile_pool(name="yps", bufs=2, space="PSUM") as yps:
            for qc in range(NC_CHUNKS):
                qs128 = slice(qc * 128, (qc + 1) * 128)
                py0 = yps.tile([128, D], F32, tag="py0")
                py1 = yps.tile([128, D], F32, tag="py1")
                nc.tensor.matmul(py0, ctx_sb[0:64, qs128], wo_sb[0:64, :],
                                 start=True, stop=True)
                nc.tensor.matmul(py1, ctx_sb[64:128, qs128], wo_sb[64:128, :],
                                 start=True, stop=True)
                y0s = yst.tile([128, D], F32, tag="y0s")
                ysb = yst.tile([128, D], F32, tag="ysb")
                nc.scalar.activation(y0s, py0, AF.Copy,
                                     scale=r_all[:, qc:qc + 1])
                nc.scalar.activation(ysb, py1, AF.Copy,
                                     scale=r_all[:, NC_CHUNKS + qc:NC_CHUNKS + qc + 1])
                nc.vector.tensor_add(ysb, ysb, y0s)
                nc.sync.dma_start(out=y[qs128, :], in_=ysb)


_MODULE_CACHE = {}


def _get_module():
    if "nc" not in _MODULE_CACHE:
        _MODULE_CACHE["nc"] = _build_module()
    return _MODULE_CACHE["nc"]


def kernel(q, k, v, mask, Wq, bq, Wk, bk, Wv, bv, Wo, bo):
    q = np.asarray(q, dtype=np.float32)
    k = np.asarray(k, dtype=np.float32)
    v = np.asarray(v, dtype=np.float32)
    mask = np.asarray(mask)
    Wq = np.asarray(Wq, dtype=np.float32)
    Wk = np.asarray(Wk, dtype=np.float32)
    Wv = np.asarray(Wv, dtype=np.float32)
    Wo = np.asarray(Wo, dtype=np.float32)
    bq = np.asarray(bq, dtype=np.float32)
    bk = np.asarray(bk, dtype=np.float32)
    bv = np.asarray(bv, dtype=np.float32)
    bo = np.asarray(bo, dtype=np.float32)

    nc = _get_module()

    f8 = ml_dtypes.float8_e5m2
    B = q.shape[0]
    qT = [np.ascontiguousarray(q[b].T) for b in range(B)]
    kT = [np.ascontiguousarray(k[b].T) for b in range(B)]
    vT = [np.ascontiguousarray(v[b].T) for b in range(B)]
    mA = []
    mT = []
    for b in range(B):
        bias = np.where(mask[b] == 0, np.float32(MASK_NEG), np.float32(0.0))
        mA.append(bias.astype(f8))
        mT.append(np.ascontiguousarray(mA[b].T))
    ident = np.eye(128, dtype=np.float32).astype(f8)

    in_maps = []
    for c in range(8):
        b = c // 4
        h0 = 2 * (c % 4)
        cs = slice(h0 * 64, h0 * 64 + 128)
        in_maps.append({
            "qT": qT[b], "kT": kT[b], "vT": vT[b],
            "maskA": mA[b], "maskT": mT[b],
            "wq": np.ascontiguousarray(Wq[:, cs]),
            "wk": np.ascontiguousarray(Wk[:, cs]),
            "wv": np.ascontiguousarray(Wv[:, cs]),
            "wo": np.ascontiguousarray(Wo[cs, :]),
            "bq": np.ascontiguousarray((bq[cs] * 0.125).reshape(128, 1)),
            "bk": np.ascontiguousarray(bk[cs].reshape(128, 1)),
            "ident": ident,
        })

    res = bass_utils.run_bass_kernel_spmd(nc, in_maps, core_ids=list(range(8)))

    attn = np.empty((B, 8, S, S), np.float32)
    out = np.zeros((B, S, D), np.float32)
    for c in range(8):
        b = c // 4
        h0 = 2 * (c % 4)
        r = res.results[c]
        attn[b, h0:h0 + 2] = r["attn2"]
        out[b] += r["y"]
    # bv contributes exactly bv @ Wo per row (attn rows sum to 1); bo on top.
    out += (bo + bv @ Wo)[None, None, :]
    return out, attn


# revision 8
# speedup vs baseline: 1.5995x; 1.5995x over previous
"""MultiHeadAttention Trainium2 kernel (8-core SPMD, full I/O).

Sharding: core c handles batch b = c // 4 and heads (2*(c%4), 2*(c%4)+1).
Each core computes:
  - attn[b, h0:h0+2]  (fp32, written directly)
  - y = partial output projection (summed on host across the 4 cores of b)

Device-side design per core (Bass/Tile):
  phase0: project qhT/khT [128, S] (head-dim on partitions, both heads) and
          vh [S, 128] from transposed inputs streamed from DRAM.
  A-pass: scores[s_q, s_k] = qhT.T @ khT (bf16 matmuls) + additive fp8 mask
          via identity-matmul into the same PSUM accumulation group;
          exp on ACT with accum_out giving row sums for free; normalize with
          per-partition reciprocal on DVE; DMA attn out.
  T-pass: transposed scores[s_k, s_q] the same way (masked with transposed
          fp8 mask); exp; ctxE^T[64, s_q] accumulated in PSUM over k-chunks.
  y-pass: y_h = (ctxE_h^T).T @ Wo_h per head, scaled by the per-row reciprocal
          (per-partition scale on ACT) and summed across the 2 heads.
Softmax skips the row-max subtraction: scores are O(1) (no overflow) and
masked entries are -10240 (exp underflows to exactly 0.0), so the result is
mathematically identical to the reference.
"""

import sys

for _p in ("/opt/trn_rl_repo", "/root/.axon_site/_ro/trn_rl_repo"):
    if _p not in sys.path:
        sys.path.append(_p)

import numpy as np
import ml_dtypes

import concourse.bass as bass
import concourse.tile as tile
from concourse import bacc, mybir
from concourse import bass_utils

F32 = mybir.dt.float32
F32R = mybir.dt.float32r
BF16 = mybir.dt.bfloat16
F8 = mybir.dt.float8e5
AF = mybir.ActivationFunctionType

S = 2048
D = 512
NC_CHUNKS = 16  # S / 128
KD = 4  # D / 128
MASK_NEG = -10240.0  # exactly representable in fp8e5m2; exp() underflows to 0


def _build_module():
    nc = bacc.Bacc("TRN2", target_bir_lowering=False, debug=False, num_devices=8)

    qT = nc.dram_tensor("qT", [D, S], BF16, kind="ExternalInput").ap()
    kT = nc.dram_tensor("kT", [D, S], BF16, kind="ExternalInput").ap()
    vT = nc.dram_tensor("vT", [D, S], BF16, kind="ExternalInput").ap()
    maskA = nc.dram_tensor("maskA", [S, S], F8, kind="ExternalInput").ap()
    maskT = nc.dram_tensor("maskT", [S, S], F8, kind="ExternalInput").ap()
    wq = nc.dram_tensor("wq", [D, 128], BF16, kind="ExternalInput").ap()
    wk = nc.dram_tensor("wk", [D, 128], BF16, kind="ExternalInput").ap()
    wv = nc.dram_tensor("wv", [D, 128], BF16, kind="ExternalInput").ap()
    wo = nc.dram_tensor("wo", [128, D], BF16, kind="ExternalInput").ap()
    bq = nc.dram_tensor("bq", [128, 1], F32, kind="ExternalInput").ap()
    bk = nc.dram_tensor("bk", [128, 1], F32, kind="ExternalInput").ap()
    ident = nc.dram_tensor("ident", [128, 128], F8, kind="ExternalInput").ap()

    attn2 = nc.dram_tensor("attn2", [2, S, S], F32, kind="ExternalOutput").ap()
    y = nc.dram_tensor("y", [S, D], F32, kind="ExternalOutput").ap()

    with tile.TileContext(nc) as tc:
        _emit(nc, tc, qT, kT, vT, maskA, maskT, wq, wk, wv, wo, bq, bk, ident,
              attn2, y)

    nc.compile()
    return nc


def _emit(nc, tc, qT, kT, vT, maskA, maskT, wq, wk, wv, wo, bq, bk, ident,
          attn2, y):
    with tc.tile_pool(name="persist", bufs=1) as persist:
        # ---- constants ----
        wq_sb = persist.tile([128, KD, 128], BF16)
        wk_sb = persist.tile([128, KD, 128], BF16)
        wv_sb = persist.tile([128, KD, 128], BF16)
        for w_sb, w in ((wq_sb, wq), (wk_sb, wk), (wv_sb, wv)):
            nc.sync.dma_start(out=w_sb, in_=w.rearrange("(k p) m -> p k m", p=128))
        wo_sb = persist.tile([128, D], BF16)
        nc.sync.dma_start(out=wo_sb, in_=wo)
        bq_sb = persist.tile([128, 1], F32)
        nc.sync.dma_start(out=bq_sb, in_=bq)
        bk_sb = persist.tile([128, 1], F32)
        nc.sync.dma_start(out=bk_sb, in_=bk)
        ident_sb = persist.tile([128, 128], F8)
        nc.sync.dma_start(out=ident_sb, in_=ident)

        # persistent activations
        qhT = persist.tile([128, S], BF16)  # both heads, head-dim on partitions
        khT = persist.tile([128, S], BF16)
        vh = persist.tile([128, NC_CHUNKS, 128], BF16)  # [s in chunk, chunk, dim]
        ctx_sb = persist.tile([128, S], BF16)  # ctxE^T both heads
        r_all = persist.tile([128, 32], F32)  # 1/rowsum per (head, q-chunk)

        # ---- phase 0: projections ----
        with tc.tile_pool(name="p0in", bufs=2) as p0in, \
             tc.tile_pool(name="p0ps", bufs=1, space="PSUM") as p0ps:
            pq = p0ps.tile([128, S], F32)  # 4 banks
            pk = p0ps.tile([128, S], F32)  # 4 banks
            for kc in range(KD):
                qt_k = p0in.tile([128, S], BF16, tag="qt")
                kt_k = p0in.tile([128, S], BF16, tag="kt")
                nc.sync.dma_start(out=qt_k, in_=qT[kc * 128:(kc + 1) * 128, :])
                nc.sync.dma_start(out=kt_k, in_=kT[kc * 128:(kc + 1) * 128, :])
                for n in range(4):
                    ns = slice(n * 512, (n + 1) * 512)
                    nc.tensor.matmul(pq[:, ns], wq_sb[:, kc, :],
                                     qt_k[:, ns],
                                     start=(kc == 0), stop=(kc == KD - 1))
                    nc.tensor.matmul(pk[:, ns], wk_sb[:, kc, :],
                                     kt_k[:, ns],
                                     start=(kc == 0), stop=(kc == KD - 1))
            for n in range(4):
                ns = slice(n * 512, (n + 1) * 512)
                # qh is pre-scaled by 1/sqrt(d_k); bq arrives pre-scaled too
                nc.scalar.activation(qhT[:, ns], pq[:, ns], AF.Identity,
                                     bias=bq_sb, scale=0.125)
                nc.scalar.activation(khT[:, ns], pk[:, ns], AF.Identity,
                                     bias=bk_sb, scale=1.0)

        with tc.tile_pool(name="p0v", bufs=3) as p0v, \
             tc.tile_pool(name="p0vps", bufs=2, space="PSUM") as p0vps:
            vT_r = vT.rearrange("(k p) (c m) -> c p k m", p=128, m=128)
            for sc in range(NC_CHUNKS):
                vt_sc = p0v.tile([128, KD, 128], BF16, tag="vt")
                nc.sync.dma_start(out=vt_sc, in_=vT_r[sc])
                pv = p0vps.tile([128, 128], F32, tag="pv")
                for kc in range(KD):
                    nc.tensor.matmul(pv, vt_sc[:, kc, :], wv_sb[:, kc, :],
                                     start=(kc == 0), stop=(kc == KD - 1))
                nc.scalar.activation(vh[:, sc, :], pv, AF.Copy)

        # ---- A pass: attn output ----
        with tc.tile_pool(name="amask", bufs=3) as amask, \
             tc.tile_pool(name="ea", bufs=2) as eap, \
             tc.tile_pool(name="attnst", bufs=2) as attnst, \
             tc.tile_pool(name="accp", bufs=2) as accp, \
             tc.tile_pool(name="aps", bufs=2, space="PSUM") as aps:
            for qc in range(NC_CHUNKS):
                qs128 = slice(qc * 128, (qc + 1) * 128)
                ma = amask.tile([128, S], F8, tag="ma")
                nc.sync.dma_start(out=ma, in_=maskA[qs128, :])
                for h in range(2):
                    hh = h * 64
                    hs = slice(hh, hh + 64)
                    pa = aps.tile([128, S], F32, tag="pa")  # 4 banks x2 bufs
                    for j in range(4):
                        ks = slice(j * 512, (j + 1) * 512)
                        nc.tensor.matmul(pa[:, ks], qhT[hs, qs128],
                                         khT[hs, ks], start=True, stop=False)
                    for j in range(4):
                        ks = slice(j * 512, (j + 1) * 512)
                        nc.tensor.matmul(pa[:, ks], ident_sb, ma[:, ks],
                                         start=False, stop=True,
                                         skip_group_check=True)
                    ea = eap.tile([128, S], F32, tag=f"ea{h}")
                    acc = accp.tile([128, 1], F32, tag=f"acc{h}")
                    nc.scalar.activation(ea, pa, AF.Exp, accum_out=acc)
                    idx = h * NC_CHUNKS + qc
                    nc.vector.reciprocal(r_all[:, idx:idx + 1], acc)
                    attn_sb = attnst.tile([128, S], F32, tag=f"attn{h}")
                    nc.vector.tensor_scalar_mul(attn_sb, ea, r_all[:, idx:idx + 1])
                    nc.sync.dma_start(out=attn2[h, qs128, :], in_=attn_sb)

        # ---- T pass: ctxE^T accumulation ----
        # Two s_q-half sweeps so everything fits in 8 PSUM banks. The ctx
        # matmul uses the full [128,128] vh as stationary operand: for head h
        # only partitions [64h, 64h+64) of the product are meaningful (the
        # other 64 are ignored), which keeps the PSUM dst partition at 0.
        with tc.tile_pool(name="tmask", bufs=3) as tmask, \
             tc.tile_pool(name="et", bufs=2) as etp, \
             tc.tile_pool(name="tps", bufs=2, space="PSUM") as tps, \
             tc.tile_pool(name="ctxps", bufs=1, space="PSUM") as ctxps:
            for half in range(2):
                hsl = slice(half * 1024, (half + 1) * 1024)
                pctx = [ctxps.tile([128, 1024], F32, tag="pctx0", name="pctx0"),
                        ctxps.tile([128, 1024], F32, tag="pctx1", name="pctx1")]
                for kc in range(NC_CHUNKS):
                    ks128 = slice(kc * 128, (kc + 1) * 128)
                    mt = tmask.tile([128, 1024], F8, tag="mt")
                    nc.sync.dma_start(out=mt, in_=maskT[ks128, hsl])
                    for h in range(2):
                        hh = h * 64
                        hs = slice(hh, hh + 64)
                        et = etp.tile([128, 1024], BF16, tag=f"et{h}")
                        pt = tps.tile([128, 1024], F32, tag="pt")  # 2 banks x2
                        for j in range(2):
                            q0 = half * 1024 + j * 512
                            qs = slice(q0, q0 + 512)
                            js = slice(j * 512, (j + 1) * 512)
                            nc.tensor.matmul(pt[:, js], khT[hs, ks128],
                                             qhT[hs, qs],
                                             start=True, stop=False)
                        for j in range(2):
                            js = slice(j * 512, (j + 1) * 512)
                            nc.tensor.matmul(pt[:, js], ident_sb, mt[:, js],
                                             start=False, stop=True,
                                             skip_group_check=True)
                        nc.scalar.activation(et, pt, AF.Exp)
                        for j in range(2):
                            js = slice(j * 512, (j + 1) * 512)
                            nc.tensor.matmul(pctx[h][:, js], vh[:, kc, :],
                                             et[:, js],
                                             start=(kc == 0),
                                             stop=(kc == NC_CHUNKS - 1),
                                             skip_group_check=True)
                nc.scalar.activation(ctx_sb[0:64, hsl], pctx[0][0:64, :], AF.Copy)
                nc.scalar.activation(ctx_sb[64:128, hsl], pctx[1][64:128, :],
                                     AF.Copy)

        # ---- y pass: output projection with deferred softmax normalization ----
        with tc.tile_pool(name="yst", bufs=3) as yst, \
             tc.tile_pool(name="yps", bufs=2, space="PSUM") as yps:
            for qc in range(NC_CHUNKS):
                qs128 = slice(qc * 128, (qc + 1) * 128)
                py0 = yps.tile([128, D], F32, tag="py0")
                py1 = yps.tile([128, D], F32, tag="py1")
                nc.tensor.matmul(py0, ctx_sb[0:64, qs128], wo_sb[0:64, :],
                                 start=True, stop=True)
                nc.tensor.matmul(py1, ctx_sb[64:128, qs128], wo_sb[64:128, :],
                                 start=True, stop=True)
                y0s = yst.tile([128, D], F32, tag="y0s")
                ysb = yst.tile([128, D], F32, tag="ysb")
                nc.scalar.activation(y0s, py0, AF.Copy,
                                     scale=r_all[:, qc:qc + 1])
                nc.scalar.activation(ysb, py1, AF.Copy,
                                     scale=r_all[:, NC_CHUNKS + qc:NC_CHUNKS + qc + 1])
                nc.vector.tensor_add(ysb, ysb, y0s)
                nc.sync.dma_start(out=y[qs128, :], in_=ysb)


_MODULE_CACHE = {}


def _get_module():
    if "nc" not in _MODULE_CACHE:
        _MODULE_CACHE["nc"] = _build_module()
    return _MODULE_CACHE["nc"]


def kernel(q, k, v, mask, Wq, bq, Wk, bk, Wv, bv, Wo, bo):
    q = np.asarray(q, dtype=np.float32)
    k = np.asarray(k, dtype=np.float32)
    v = np.asarray(v, dtype=np.float32)
    mask = np.asarray(mask)
    Wq = np.asarray(Wq, dtype=np.float32)
    Wk = np.asarray(Wk, dtype=np.float32)
    Wv = np.asarray(Wv, dtype=np.float32)
    Wo = np.asarray(Wo, dtype=np.float32)
    bq = np.asarray(bq, dtype=np.float32)
    bk = np.asarray(bk, dtype=np.float32)
    bv = np.asarray(bv, dtype=np.float32)
    bo = np.asarray(bo, dtype=np.float32)

    nc = _get_module()
    in_maps = _make_in_maps(q, k, v, mask, Wq, bq, Wk, bk, Wv, bv, Wo, bo)

    res = bass_utils.run_bass_kernel_spmd(nc, in_maps, core_ids=list(range(8)))
    return _gather(res.results, q.shape[0], bq, bv, bo, Wo)


def _make_in_maps(q, k, v, mask, Wq, bq, Wk, bk, Wv, bv, Wo, bo):
    f8 = ml_dtypes.float8_e5m2
    B = q.shape[0]
    bf16 = ml_dtypes.bfloat16
    qT = [np.ascontiguousarray(q[b].T).astype(bf16) for b in range(B)]
    kT = [np.ascontiguousarray(k[b].T).astype(bf16) for b in range(B)]
    vT = [np.ascontiguousarray(v[b].T).astype(bf16) for b in range(B)]
    mA = []
    mT = []
    for b in range(B):
        bias = np.where(mask[b] == 0, np.float32(MASK_NEG), np.float32(0.0))
        mA.append(bias.astype(f8))
        mT.append(np.ascontiguousarray(mA[b].T))
    ident = np.eye(128, dtype=np.float32).astype(f8)

    in_maps = []
    for c in range(8):
        b = c // 4
        h0 = 2 * (c % 4)
        cs = slice(h0 * 64, h0 * 64 + 128)
        in_maps.append({
            "qT": qT[b], "kT": kT[b], "vT": vT[b],
            "maskA": mA[b], "maskT": mT[b],
            "wq": np.ascontiguousarray(Wq[:, cs]).astype(bf16),
            "wk": np.ascontiguousarray(Wk[:, cs]).astype(bf16),
            "wv": np.ascontiguousarray(Wv[:, cs]).astype(bf16),
            "wo": np.ascontiguousarray(Wo[cs, :]).astype(bf16),
            "bq": np.ascontiguousarray((bq[cs] * 0.125).reshape(128, 1)),
            "bk": np.ascontiguousarray(bk[cs].reshape(128, 1)),
            "ident": ident,
        })
    return in_maps


def _gather(results, B, bq, bv, bo, Wo):
    attn = np.empty((B, 8, S, S), np.float32)
    out = np.zeros((B, S, D), np.float32)
    for c in range(8):
        b = c // 4
        h0 = 2 * (c % 4)
        r = results[c]
        attn[b, h0:h0 + 2] = r["attn2"]
        out[b] += r["y"]
    # bv contributes exactly bv @ Wo per row (attn rows sum to 1); bo on top.
    out += (bo + bv @ Wo)[None, None, :]
    return out, attn


# revision 9
# speedup vs baseline: 1.8235x; 1.1400x over previous
"""MultiHeadAttention Trainium2 kernel (8-core SPMD, full I/O).

Sharding: core c handles batch b = c // 4 and heads (2*(c%4), 2*(c%4)+1).
Each core computes:
  - attn[b, h0:h0+2]  (fp32, written directly)
  - y = partial output projection (summed on host across the 4 cores of b)

Device-side design per core (Bass/Tile):
  phase0: project qhT/khT [128, S] (head-dim on partitions, both heads) and
          vh [S, 128] from transposed inputs streamed from DRAM.
  A-pass: scores[s_q, s_k] = qhT.T @ khT (bf16 matmuls) + additive fp8 mask
          via identity-matmul into the same PSUM accumulation group;
          exp on ACT with accum_out giving row sums for free; normalize with
          per-partition reciprocal on DVE; DMA attn out.
  T-pass: transposed scores[s_k, s_q] the same way (masked with transposed
          fp8 mask); exp; ctxE^T[64, s_q] accumulated in PSUM over k-chunks.
  y-pass: y_h = (ctxE_h^T).T @ Wo_h per head, scaled by the per-row reciprocal
          (per-partition scale on ACT) and summed across the 2 heads.
Softmax skips the row-max subtraction: scores are O(1) (no overflow) and
masked entries are -10240 (exp underflows to exactly 0.0), so the result is
mathematically identical to the reference.
"""

import sys

for _p in ("/opt/trn_rl_repo", "/root/.axon_site/_ro/trn_rl_repo"):
    if _p not in sys.path:
        sys.path.append(_p)

import numpy as np
import ml_dtypes

import concourse.bass as bass
import concourse.tile as tile
from concourse import bacc, mybir
from concourse import bass_utils

F32 = mybir.dt.float32
F32R = mybir.dt.float32r
BF16 = mybir.dt.bfloat16
F8 = mybir.dt.float8e5
AF = mybir.ActivationFunctionType

S = 2048
D = 512
NC_CHUNKS = 16  # S / 128
KD = 4  # D / 128
MASK_NEG = -10240.0  # exactly representable in fp8e5m2; exp() underflows to 0


def _build_module():
    nc = bacc.Bacc("TRN2", target_bir_lowering=False, debug=False, num_devices=8)

    qT = nc.dram_tensor("qT", [D, S], BF16, kind="ExternalInput").ap()
    kT = nc.dram_tensor("kT", [D, S], BF16, kind="ExternalInput").ap()
    vT = nc.dram_tensor("vT", [D, S], BF16, kind="ExternalInput").ap()
    maskA = nc.dram_tensor("maskA", [S, S], F8, kind="ExternalInput").ap()
    maskT = nc.dram_tensor("maskT", [S, S], F8, kind="ExternalInput").ap()
    wq = nc.dram_tensor("wq", [D, 128], BF16, kind="ExternalInput").ap()
    wk = nc.dram_tensor("wk", [D, 128], BF16, kind="ExternalInput").ap()
    wv = nc.dram_tensor("wv", [D, 128], BF16, kind="ExternalInput").ap()
    wo = nc.dram_tensor("wo", [128, D], BF16, kind="ExternalInput").ap()
    bq = nc.dram_tensor("bq", [128, 1], F32, kind="ExternalInput").ap()
    bk = nc.dram_tensor("bk", [128, 1], F32, kind="ExternalInput").ap()
    ident = nc.dram_tensor("ident", [128, 128], F8, kind="ExternalInput").ap()

    attn2 = nc.dram_tensor("attn2", [2, S, S], F32, kind="ExternalOutput").ap()
    y = nc.dram_tensor("y", [S, D], F32, kind="ExternalOutput").ap()

    with tile.TileContext(nc) as tc:
        _emit(nc, tc, qT, kT, vT, maskA, maskT, wq, wk, wv, wo, bq, bk, ident,
              attn2, y)

    nc.compile()
    return nc


def _emit(nc, tc, qT, kT, vT, maskA, maskT, wq, wk, wv, wo, bq, bk, ident,
          attn2, y):
    with tc.tile_pool(name="persist", bufs=1) as persist:
        # ---- constants ----
        wq_sb = persist.tile([128, KD, 128], BF16)
        wk_sb = persist.tile([128, KD, 128], BF16)
        wv_sb = persist.tile([128, KD, 128], BF16)
        for w_sb, w in ((wq_sb, wq), (wk_sb, wk), (wv_sb, wv)):
            nc.sync.dma_start(out=w_sb, in_=w.rearrange("(k p) m -> p k m", p=128))
        wo_sb = persist.tile([128, D], BF16)
        nc.sync.dma_start(out=wo_sb, in_=wo)
        bq_sb = persist.tile([128, 1], F32)
        nc.sync.dma_start(out=bq_sb, in_=bq)
        bk_sb = persist.tile([128, 1], F32)
        nc.sync.dma_start(out=bk_sb, in_=bk)
        ident_sb = persist.tile([128, 128], F8)
        nc.sync.dma_start(out=ident_sb, in_=ident)

        # persistent activations
        qhT = persist.tile([128, S], BF16)  # both heads, head-dim on partitions
        khT = persist.tile([128, S], BF16)
        # vh with the other head's columns zeroed, so both heads can
        # accumulate into one shared ctx PSUM tile (zero half adds +0).
        vh0 = persist.tile([128, NC_CHUNKS, 128], BF16)
        vh1 = persist.tile([128, NC_CHUNKS, 128], BF16)
        nc.gpsimd.memset(vh0, 0.0)
        nc.gpsimd.memset(vh1, 0.0)
        ctx_sb = persist.tile([128, S], BF16)  # ctxE^T both heads
        r_all = persist.tile([128, 32], F32)  # 1/rowsum per (head, q-chunk)

        # ---- phase 0: projections ----
        with tc.tile_pool(name="p0in", bufs=2) as p0in, \
             tc.tile_pool(name="p0ps", bufs=1, space="PSUM") as p0ps:
            pq = p0ps.tile([128, S], F32)  # 4 banks
            pk = p0ps.tile([128, S], F32)  # 4 banks
            for kc in range(KD):
                qt_k = p0in.tile([128, S], BF16, tag="qt")
                kt_k = p0in.tile([128, S], BF16, tag="kt")
                nc.sync.dma_start(out=qt_k, in_=qT[kc * 128:(kc + 1) * 128, :])
                nc.sync.dma_start(out=kt_k, in_=kT[kc * 128:(kc + 1) * 128, :])
                for n in range(4):
                    ns = slice(n * 512, (n + 1) * 512)
                    nc.tensor.matmul(pq[:, ns], wq_sb[:, kc, :],
                                     qt_k[:, ns],
                                     start=(kc == 0), stop=(kc == KD - 1))
                    nc.tensor.matmul(pk[:, ns], wk_sb[:, kc, :],
                                     kt_k[:, ns],
                                     start=(kc == 0), stop=(kc == KD - 1))
            for n in range(4):
                ns = slice(n * 512, (n + 1) * 512)
                # qh is pre-scaled by 1/sqrt(d_k); bq arrives pre-scaled too
                nc.scalar.activation(qhT[:, ns], pq[:, ns], AF.Identity,
                                     bias=bq_sb, scale=0.125)
                nc.scalar.activation(khT[:, ns], pk[:, ns], AF.Identity,
                                     bias=bk_sb, scale=1.0)

        with tc.tile_pool(name="p0v", bufs=3) as p0v, \
             tc.tile_pool(name="p0vps", bufs=2, space="PSUM") as p0vps:
            vT_r = vT.rearrange("(k p) (c m) -> c p k m", p=128, m=128)
            for sc in range(NC_CHUNKS):
                vt_sc = p0v.tile([128, KD, 128], BF16, tag="vt")
                nc.sync.dma_start(out=vt_sc, in_=vT_r[sc])
                pv = p0vps.tile([128, 128], F32, tag="pv")
                for kc in range(KD):
                    nc.tensor.matmul(pv, vt_sc[:, kc, :], wv_sb[:, kc, :],
                                     start=(kc == 0), stop=(kc == KD - 1))
                nc.scalar.activation(vh0[:, sc, 0:64], pv[:, 0:64], AF.Copy)
                nc.scalar.activation(vh1[:, sc, 64:128], pv[:, 64:128], AF.Copy)

        # ---- main loop: A (attn out) and T (ctx accum) interleaved ----
        # PSUM: shared score pool "ps" [128,1024] x2 bufs = 4 banks,
        # ctx accumulator [128, 2048] = 4 banks.
        with tc.tile_pool(name="amask", bufs=3) as amask, \
             tc.tile_pool(name="tmask", bufs=3) as tmask, \
             tc.tile_pool(name="ea", bufs=2) as eap, \
             tc.tile_pool(name="et", bufs=3) as etp, \
             tc.tile_pool(name="attnst", bufs=2) as attnst, \
             tc.tile_pool(name="accp", bufs=2) as accp, \
             tc.tile_pool(name="sps", bufs=2, space="PSUM") as sps, \
             tc.tile_pool(name="ctxps", bufs=1, space="PSUM") as ctxps:
            pctx = ctxps.tile([128, S], F32)  # 4 banks, shared by both heads
            vhz = (vh0, vh1)
            for i in range(NC_CHUNKS):
                # ---- A chunk: attn rows [i*128, (i+1)*128) ----
                qc = i
                qs128 = slice(qc * 128, (qc + 1) * 128)
                ma = amask.tile([128, S], F8, tag="ma")
                nc.sync.dma_start(out=ma, in_=maskA[qs128, :])
                for h in range(2):
                    hh = h * 64
                    hs = slice(hh, hh + 64)
                    ea = eap.tile([128, S], F32, tag=f"ea{h}")
                    acc = accp.tile([128, 2], F32, tag=f"acc{h}")
                    for half in range(2):
                        pa = sps.tile([128, 1024], F32, tag="ps")
                        for j in range(2):
                            ks = slice(half * 1024 + j * 512,
                                       half * 1024 + (j + 1) * 512)
                            nc.tensor.matmul(pa[:, j * 512:(j + 1) * 512],
                                             qhT[hs, qs128], khT[hs, ks],
                                             start=True, stop=False)
                        for j in range(2):
                            ks = slice(half * 1024 + j * 512,
                                       half * 1024 + (j + 1) * 512)
                            nc.tensor.matmul(pa[:, j * 512:(j + 1) * 512],
                                             ident_sb, ma[:, ks],
                                             start=False, stop=True,
                                             skip_group_check=True)
                        nc.scalar.activation(
                            ea[:, half * 1024:(half + 1) * 1024], pa, AF.Exp,
                            accum_out=acc[:, half:half + 1])
                    idx = h * NC_CHUNKS + qc
                    rsum = accp.tile([128, 1], F32, tag=f"rsum{h}")
                    nc.vector.tensor_add(rsum, acc[:, 0:1], acc[:, 1:2])
                    nc.vector.reciprocal(r_all[:, idx:idx + 1], rsum)
                    attn_sb = attnst.tile([128, S], F32, tag=f"attn{h}")
                    nc.vector.tensor_scalar_mul(attn_sb, ea,
                                                r_all[:, idx:idx + 1])
                    nc.sync.dma_start(out=attn2[h, qs128, :], in_=attn_sb)

                # ---- T chunk: ctx contribution of k rows [i*128, ...) ----
                kc = i
                ks128 = slice(kc * 128, (kc + 1) * 128)
                mt = tmask.tile([128, S], F8, tag="mt")
                nc.sync.dma_start(out=mt, in_=maskT[ks128, :])
                for h in range(2):
                    hh = h * 64
                    hs = slice(hh, hh + 64)
                    et = etp.tile([128, S], BF16, tag=f"et{h}")
                    for half in range(2):
                        pt = sps.tile([128, 1024], F32, tag="ps")
                        for j in range(2):
                            qs = slice(half * 1024 + j * 512,
                                       half * 1024 + (j + 1) * 512)
                            nc.tensor.matmul(pt[:, j * 512:(j + 1) * 512],
                                             khT[hs, ks128], qhT[hs, qs],
                                             start=True, stop=False)
                        for j in range(2):
                            qs = slice(half * 1024 + j * 512,
                                       half * 1024 + (j + 1) * 512)
                            nc.tensor.matmul(pt[:, j * 512:(j + 1) * 512],
                                             ident_sb, mt[:, qs],
                                             start=False, stop=True,
                                             skip_group_check=True)
                        nc.scalar.activation(
                            et[:, half * 1024:(half + 1) * 1024], pt, AF.Exp)
                    for j in range(4):
                        qs = slice(j * 512, (j + 1) * 512)
                        nc.tensor.matmul(pctx[:, qs], vhz[h][:, kc, :],
                                         et[:, qs],
                                         start=(i == 0 and h == 0),
                                         stop=(i == NC_CHUNKS - 1 and h == 1),
                                         skip_group_check=True)
            nc.scalar.activation(ctx_sb, pctx, AF.Copy)

        # ---- y pass: output projection with deferred softmax normalization ----
        with tc.tile_pool(name="yst", bufs=3) as yst, \
             tc.tile_pool(name="yps", bufs=2, space="PSUM") as yps:
            for qc in range(NC_CHUNKS):
                qs128 = slice(qc * 128, (qc + 1) * 128)
                py0 = yps.tile([128, D], F32, tag="py0")
                py1 = yps.tile([128, D], F32, tag="py1")
                nc.tensor.matmul(py0, ctx_sb[0:64, qs128], wo_sb[0:64, :],
                                 start=True, stop=True)
                nc.tensor.matmul(py1, ctx_sb[64:128, qs128], wo_sb[64:128, :],
                                 start=True, stop=True)
                y0s = yst.tile([128, D], F32, tag="y0s")
                ysb = yst.tile([128, D], F32, tag="ysb")
                nc.vector.tensor_scalar_mul(y0s, py0, r_all[:, qc:qc + 1])
                nc.vector.tensor_scalar_mul(
                    ysb, py1, r_all[:, NC_CHUNKS + qc:NC_CHUNKS + qc + 1])
                nc.vector.tensor_add(ysb, ysb, y0s)
                nc.sync.dma_start(out=y[qs128, :], in_=ysb)


_MODULE_CACHE = {}


def _get_module():
    if "nc" not in _MODULE_CACHE:
        _MODULE_CACHE["nc"] = _build_module()
    return _MODULE_CACHE["nc"]


def kernel(q, k, v, mask, Wq, bq, Wk, bk, Wv, bv, Wo, bo):
    q = np.asarray(q, dtype=np.float32)
    k = np.asarray(k, dtype=np.float32)
    v = np.asarray(v, dtype=np.float32)
    mask = np.asarray(mask)
    Wq = np.asarray(Wq, dtype=np.float32)
    Wk = np.asarray(Wk, dtype=np.float32)
    Wv = np.asarray(Wv, dtype=np.float32)
    Wo = np.asarray(Wo, dtype=np.float32)
    bq = np.asarray(bq, dtype=np.float32)
    bk = np.asarray(bk, dtype=np.float32)
    bv = np.asarray(bv, dtype=np.float32)
    bo = np.asarray(bo, dtype=np.float32)

    nc = _get_module()
    in_maps = _make_in_maps(q, k, v, mask, Wq, bq, Wk, bk, Wv, bv, Wo, bo)

    res = bass_utils.run_bass_kernel_spmd(nc, in_maps, core_ids=list(range(8)))
    return _gather(res.results, q.shape[0], bq, bv, bo, Wo)


def _make_in_maps(q, k, v, mask, Wq, bq, Wk, bk, Wv, bv, Wo, bo):
    f8 = ml_dtypes.float8_e5m2
    B = q.shape[0]
    bf16 = ml_dtypes.bfloat16
    qT = [np.ascontiguousarray(q[b].T).astype(bf16) for b in range(B)]
    kT = [np.ascontiguousarray(k[b].T).astype(bf16) for b in range(B)]
    vT = [np.ascontiguousarray(v[b].T).astype(bf16) for b in range(B)]
    mA = []
    mT = []
    for b in range(B):
        bias = np.where(mask[b] == 0, np.float32(MASK_NEG), np.float32(0.0))
        mA.append(bias.astype(f8))
        mT.append(np.ascontiguousarray(mA[b].T))
    ident = np.eye(128, dtype=np.float32).astype(f8)

    in_maps = []
    for c in range(8):
        b = c // 4
        h0 = 2 * (c % 4)
        cs = slice(h0 * 64, h0 * 64 + 128)
        in_maps.append({
            "qT": qT[b], "kT": kT[b], "vT": vT[b],
            "maskA": mA[b], "maskT": mT[b],
            "wq": np.ascontiguousarray(Wq[:, cs]).astype(bf16),
            "wk": np.ascontiguousarray(Wk[:, cs]).astype(bf16),
            "wv": np.ascontiguousarray(Wv[:, cs]).astype(bf16),
            "wo": np.ascontiguousarray(Wo[cs, :]).astype(bf16),
            "bq": np.ascontiguousarray((bq[cs] * 0.125).reshape(128, 1)),
            "bk": np.ascontiguousarray(bk[cs].reshape(128, 1)),
            "ident": ident,
        })
    return in_maps


def _gather(results, B, bq, bv, bo, Wo):
    attn = np.empty((B, 8, S, S), np.float32)
    out = np.zeros((B, S, D), np.float32)
    for c in range(8):
        b = c // 4
        h0 = 2 * (c % 4)
        r = results[c]
        attn[b, h0:h0 + 2] = r["attn2"]
        out[b] += r["y"]
    # bv contributes exactly bv @ Wo per row (attn rows sum to 1); bo on top.
    out += (bo + bv @ Wo)[None, None, :]
    return out, attn
